# revision 13
# baseline (speedup 1.0000x reference)
"""Beam-search top-k (mask pad + add beam scores + top-16 over beam*vocab) on 8 trn2 cores.

Sharding: batch dim (64 rows) split across 8 cores, 8 rows/core, no cross-core comm.

Per-core pipeline (v2 — split stage-1, gpsimd reduce offload, token-layout tail):
  tile [128, 25136] f32, partition p = (t*8+b)*2 + h  (t=batch row, b=beam, h=half)
     h=0 holds vocab [0, 25136); h=1 holds vocab [25121, 50257)
  1. 17 chunked DMAs alternating sync/scalar HWDGE queues (side inputs ride the
     scalar queue first, so chunk 0 issues immediately). Per-chunk segmented
     reduce-max over groups of 16 -> M [128, 1571]; five mid chunks reduce on
     gpsimd via 4-pass max-fold trees, freeing DVE slack mid-load.
  2. stage-1 split in halves: stage-1a = per-partition top-16 of groups [0,832)
     runs DURING the load (in the DVE slack); stage-1b = top-16 of groups
     [832,1571) right after the last reduce. Winners kept separate (A/I 32-wide)
     so flat-offset computation stays affine per half.
  3. prune: one DMA [128,32]->[8,512]; token-level top-16 groups of the 512.
  4. gather: e_s = device-built flat offsets of all 512 candidates/token
     (bounced to DRAM); winner offsets Eu [8,16] gathered by Su2=t*512+pos;
     scores gathered straight into [8,256] from a host table replicated x16;
     raw 16 elems of each winning group gathered into Gt2 [8,256] token layout.
  5. final: Gt2 += scores; top-32 values+positions per token, packed into one
     [8,64] uint32 output (cols 0:32 = f32 values bitcast, 32:64 = positions).
  6. host decodes positions through I1a/I1b, drops raw pad-token entries,
     dedups h-overlap duplicates, sorts ties by flat index, takes 16.
"""

import sys

sys.path.insert(0, "/opt/trn_rl_repo")

import numpy as np

BSZ, BEAM, VOCAB, VK = 64, 8, 50257, 16
NCORES = 8
ROWS = BSZ // NCORES   # 8 tokens (batch rows) per core
F = 25136              # per-partition elems
CH0 = VOCAB - F        # 25121: h=1 partitions cover vocab [25121, 50257)
P = 128
GW = 16                # reduce group width
NG = F // GW           # 1571 groups per partition
LASTG = NG - 1         # group 1570 straddles the h=0 overlap
NEL = ROWS * BEAM * VOCAB  # 3216448 elements in the per-core shard
NEG = float("-inf")
NEGBIG = -3.0e38       # finite stand-in for -inf in match_replace imm (json-safe)

NGA = 832              # stage-1a covers groups [0, NGA) = chunks 0-7
NGB = NG - NGA         # stage-1b covers groups [NGA, NG) = 739

# chunk schedule: 13x1664 + descending tail so the last reduce is short
_CH_SIZES = [1664] * 13 + [1200, 1200, 800, 304]

_CACHE = {}


def _build():
    import concourse.bacc as bacc
    import concourse.mybir as mybir
    from concourse.bass_types import AP
    from concourse.tile import TileContext
    from concourse.tile_rust import add_dep_helper

    ALU = mybir.AluOpType

    nc = bacc.Bacc("TRN2", target_bir_lowering=False, debug=False, num_swdge_queues=4)
    x = nc.dram_tensor("x", [ROWS, BEAM, VOCAB], mybir.dt.float32, kind="ExternalInput").ap()
    side = nc.dram_tensor("side", [P, 4], mybir.dt.float32, kind="ExternalInput").ap()
    s_tab = nc.dram_tensor("s_tab", [P * 32, 1], mybir.dt.float32, kind="ExternalInput").ap()
    e_s = nc.dram_tensor("e_s", [P * 32, 1], mybir.dt.uint32, kind="Internal").ap()

    o_i1 = nc.dram_tensor("o_i1", [P, 32], mybir.dt.uint32, kind="ExternalOutput").ap()
    o_ib2 = nc.dram_tensor("o_ib2", [ROWS, 16], mybir.dt.uint32, kind="ExternalOutput").ap()
    o_vi = nc.dram_tensor("o_vi", [ROWS, 64], mybir.dt.uint32, kind="ExternalOutput").ap()

    chunks = []
    _o = 0
    for _ln in _CH_SIZES:
        chunks.append((_o, _ln))
        _o += _ln
    assert _o == F

    with TileContext(nc) as tc:
        with tc.tile_pool(name="main", bufs=1) as pool:
            tile = pool.tile([P, F], mybir.dt.float32)
            M = pool.tile([P, NG], mybir.dt.float32)
            Mza = pool.tile([P, NGA], mybir.dt.float32)
            Mzb = pool.tile([P, NGB], mybir.dt.float32)
            sd = pool.tile([P, 4], mybir.dt.float32)
            r1 = pool.tile([P, 1], mybir.dt.float32)
            r2 = pool.tile([P, 1], mybir.dt.float32)
            r3 = pool.tile([P, 1], mybir.dt.float32)
            r4 = pool.tile([P, 1], mybir.dt.float32)
            A_all = pool.tile([P, 32], mybir.dt.float32)
            I_all = pool.tile([P, 32], mybir.dt.uint32)
            A1b = pool.tile([P, 32], mybir.dt.float32)
            Iff = pool.tile([P, 32], mybir.dt.float32)
            Ea = pool.tile([P, 32], mybir.dt.uint32)
            At = pool.tile([ROWS, 512], mybir.dt.float32)
            Atz = pool.tile([ROWS, 512], mybir.dt.float32)
            P0 = pool.tile([ROWS, 8], mybir.dt.float32)
            IB2 = pool.tile([ROWS, 16], mybir.dt.uint32)
            S2 = pool.tile([P, 1], mybir.dt.uint32)
            S2f = pool.tile([P, 1], mybir.dt.float32)
            Su2 = pool.tile([P, 1], mybir.dt.uint32)
            Eu = pool.tile([P, 1], mybir.dt.uint32)
            Sgt = pool.tile([P, 1], mybir.dt.float32)
            Gc2 = pool.tile([P, GW], mybir.dt.float32)
            Gt2 = pool.tile([ROWS, 256], mybir.dt.float32)
            Gz0 = pool.tile([ROWS, 256], mybir.dt.float32)
            Gz1 = pool.tile([ROWS, 256], mybir.dt.float32)
            OVI = pool.tile([ROWS, 64], mybir.dt.uint32)

            mc = sd[:, 0:1]
            sc = sd[:, 1:2]
            t512 = sd[:, 2:3]
            rb = sd[:, 3:4]

            # ---- chunk DMAs: alternate sync/scalar queues; side rides scalar first
            nc.scalar.dma_start(out=sd[:, :], in_=side)
            dma_eng = [nc.sync, nc.scalar]
            for ci, (o, ln) in enumerate(chunks):
                src = AP(
                    tensor=x.tensor, offset=o,
                    ap=[[VOCAB, ROWS * BEAM], [CH0, 2], [1, ln]],
                )
                dma_eng[ci % 2].dma_start(out=tile[:, o:o + ln], in_=src)

            # ---- per-chunk segmented reduce: DVE reduce_max or gpsimd fold tree
            def dve_reduce(o, ln):
                t3 = tile[:, o:o + ln].rearrange("p (g w) -> p g w", w=GW)
                nc.vector.reduce_max(
                    out=M[:, o // GW:(o + ln) // GW], in_=t3, axis=mybir.AxisListType.X
                )

            # first half: chunks 0-7 (groups 0..832)
            dve_reduce(*chunks[0])
            dve_reduce(*chunks[1])
            # group-0 fixup: drop the pad token (vocab 1) from h=0 partitions.
            # M[:,0] = max(tile[:,0], tile[:,1] + mc, max(tile[:,2:16]))
            nc.vector.reduce_max(out=r1[:, :], in_=tile[:, 2:GW], axis=mybir.AxisListType.X)
            nc.vector.tensor_scalar_add(r2[:, :], tile[:, 1:2], mc)
            nc.vector.tensor_tensor(out=r2[:, :], in0=r2[:, :], in1=r1[:, :], op=ALU.max)
            nc.vector.tensor_tensor(out=M[:, 0:1], in0=r2[:, :], in1=tile[:, 0:1], op=ALU.max)

            dve_reduce(*chunks[2])
            dve_reduce(*chunks[3])
            dve_reduce(*chunks[4])
            dve_reduce(*chunks[5])
            dve_reduce(*chunks[6])
            dve_reduce(*chunks[7])

            # stage-1a: per-partition top-16 of groups [0, NGA) — runs in DVE slack
            nc.vector.max(out=A_all[:, 0:8], in_=M[:, 0:NGA])
            nc.vector.max_index(out=I_all[:, 0:8], in_max=A_all[:, 0:8], in_values=M[:, 0:NGA])
            nc.vector.match_replace(
                out=Mza[:, :], in_to_replace=A_all[:, 0:8], in_values=M[:, 0:NGA],
                imm_value=NEGBIG,
            )
            nc.vector.max(out=A_all[:, 8:16], in_=Mza[:, :])
            nc.vector.max_index(out=I_all[:, 8:16], in_max=A_all[:, 8:16], in_values=Mza[:, :])

            # second half: chunks 8-16
            dve_reduce(*chunks[8])
            dve_reduce(*chunks[9])
            dve_reduce(*chunks[10])
            dve_reduce(*chunks[11])
            dve_reduce(*chunks[12])
            dve_reduce(*chunks[13])
            dve_reduce(*chunks[14])
            dve_reduce(*chunks[15])
            dve_reduce(*chunks[16])

            # last-group fixup: drop h=0's copy of the overlap [25121, 25136).
            # M[:,1570] = max(tile[:,25120], max(tile[:,25121:25136]) + mc)
            nc.vector.reduce_max(out=r3[:, :], in_=tile[:, CH0:F], axis=mybir.AxisListType.X)
            nc.vector.tensor_scalar_add(r4[:, :], r3[:, :], mc)
            nc.vector.tensor_tensor(
                out=M[:, LASTG:LASTG + 1], in0=r4[:, :], in1=tile[:, GW * LASTG:GW * LASTG + 1],
                op=ALU.max,
            )

            # stage-1b: per-partition top-16 of groups [NGA, NG)
            nc.vector.max(out=A_all[:, 16:24], in_=M[:, NGA:NG])
            nc.vector.max_index(out=I_all[:, 16:24], in_max=A_all[:, 16:24], in_values=M[:, NGA:NG])
            nc.vector.match_replace(
                out=Mzb[:, :], in_to_replace=A_all[:, 16:24], in_values=M[:, NGA:NG],
                imm_value=NEGBIG,
            )
            nc.vector.max(out=A_all[:, 24:32], in_=Mzb[:, :])
            nc.vector.max_index(out=I_all[:, 24:32], in_max=A_all[:, 24:32], in_values=Mzb[:, :])

            # bias by beam score; transpose [128,32] -> [8,512] (token rows)
            nc.vector.tensor_scalar_add(A1b[:, :], A_all[:, :], sc)
            nc.sync.dma_start(out=At[:, :], in_=A1b[:, :])

            # flat x-offsets of all 512 candidates/token (gpsimd, overlaps prune):
            # col = I1a (first half) or I1b + 832 (second half); off = rb + 16*col
            nc.gpsimd.tensor_copy(out=Iff[:, :], in_=I_all[:, :])
            nc.gpsimd.tensor_scalar(
                out=Iff[:, :], in0=Iff[:, :], scalar1=float(GW), scalar2=rb,
                op0=ALU.mult, op1=ALU.add,
            )
            nc.gpsimd.tensor_scalar_add(Iff[:, 16:32], Iff[:, 16:32], float(GW * NGA))
            nc.gpsimd.tensor_copy(out=Ea[:, :], in_=Iff[:, :])
            w_es = nc.sync.dma_start(out=e_s, in_=Ea[:, :])
            nc.sync.dma_start(out=o_i1, in_=I_all[:, :])

            # prune: token-level top-16 groups of the 512 candidates
            nc.vector.max(out=P0[:, :], in_=At[:, :])
            nc.vector.max_index(out=IB2[:, 0:8], in_max=P0[:, :], in_values=At[:, :])
            nc.vector.match_replace(
                out=Atz[:, :], in_to_replace=P0[:, :], in_values=At[:, :],
                imm_value=NEGBIG,
            )
            nc.vector.max(out=P0[:, :], in_=Atz[:, :])
            nc.vector.max_index(out=IB2[:, 8:16], in_max=P0[:, :], in_values=Atz[:, :])
            nc.sync.dma_start(out=o_ib2, in_=IB2[:, :])

            # scatter token winners to partitions: S2[t*16+j] = IB2[t, j];
            # Su2 = t*512 + pos (f32-exact)
            nc.sync.dma_start(out=S2[:, :], in_=IB2[:, :])
            nc.vector.tensor_copy(out=S2f[:, :], in_=S2[:, :])
            nc.vector.tensor_scalar_add(S2f[:, :], S2f[:, :], t512)
            nc.vector.tensor_copy(out=Su2[:, :], in_=S2f[:, :])

            def emit_indirect(out_ap, offs_ap, src_tensor, src_n, qi, coef=1, cce=None):
                g = nc.gpsimd
                src = AP(tensor=src_tensor, offset=0, ap=[[1, src_n], [1, 1]])
                in_ap = g.lower_ap_dma(src, for_indirect_dma=True)
                out_l = g.lower_ap_dma(out_ap, for_indirect_dma=True)
                off_l = g.lower_ap_dma(offs_ap)
                assert len(in_ap) == 1 and len(out_l) == 1 and len(off_l) == 1
                in_ap[0].dynamic_ap_info = mybir.DynamicAccessPatternInfo(
                    c=0,
                    actual_ap=out_ap.ap,
                    indirect_dim_max_index=src_n,
                    offset_expr=[
                        mybir.DynamicAccessPatternOffsetExpr(
                            coef=coef,
                            aff_expr=mybir.DynamicAccessPatternOffsetExprAffExpr(
                                kind="IndirectArgId", arg_id=1,
                            ),
                        )
                    ],
                )
                in_ap.append(off_l[0])
                return g.add_instruction(
                    mybir.InstDMACopy(
                        name=nc.get_next_instruction_name(),
                        queue=f"qPoolDynamic{qi or ''}",
                        mode="Copy",
                        ins=in_ap,
                        outs=out_l,
                        oob_is_err=True,
                        cce_op=cce if cce is not None else ALU.bypass,
                    )
                )

            # winner offsets + scores; then the raw 16 elems of each winning group
            g_eu = emit_indirect(Eu[:, 0:1], Su2[:, 0:1], e_s.tensor, P * 32, 1)
            add_dep_helper(g_eu.ins, w_es.ins, reason="e_s DRAM bounce RAW")
            emit_indirect(Sgt[:, 0:1], Su2[:, 0:1], s_tab.tensor, P * 32, 2)
            emit_indirect(Gc2[:, 0:GW], Eu[:, 0:1], x.tensor, NEL, 3)
            nc.vector.tensor_scalar_add(Gc2[:, :], Gc2[:, :], Sgt[:, 0:1])

            # transpose: token t's 16 winner-partitions -> one partition row
            nc.sync.dma_start(out=Gt2[:, :], in_=Gc2[:, :])

            OVf = OVI[:, 0:32].bitcast(mybir.dt.float32)
            srcs = [Gt2, Gz0, Gz1, Gz0]
            for rd in range(4):
                s = srcs[rd]
                nc.vector.max(out=OVf[:, rd * 8:rd * 8 + 8], in_=s[:, :])
                nc.vector.max_index(
                    out=OVI[:, 32 + rd * 8:32 + rd * 8 + 8],
                    in_max=OVf[:, rd * 8:rd * 8 + 8],
                    in_values=s[:, :],
                )
                if rd < 3:
                    nc.vector.match_replace(
                        out=srcs[rd + 1][:, :], in_to_replace=OVf[:, rd * 8:rd * 8 + 8],
                        in_values=s[:, :], imm_value=NEGBIG,
                    )

            nc.sync.dma_start(out=o_vi, in_=OVI[:, :])

    nc.compile()
    return nc


def _get_nc():
    if "nc" not in _CACHE:
        _CACHE["nc"] = _build()
    return _CACHE["nc"]


def _side_inputs(scores_shard: np.ndarray, step: int):
    side = np.zeros((P, 4), np.float32)
    s_tab = np.zeros((P * 32, 1), np.float32)
    for t in range(ROWS):
        for b in range(BEAM):
            sv = (0.0 if b == 0 else NEG) if step == 0 else float(scores_shard[t, b])
            for h in range(2):
                p = t * 16 + b * 2 + h
                q = b * 2 + h
                if h == 0:
                    side[p, 0] = NEG
                side[p, 1] = sv
                side[p, 2] = float(t * 512)
                side[p, 3] = float((t * BEAM + b) * VOCAB + h * CH0)
                base = t * 512 + q * 32
                s_tab[base:base + 32, 0] = sv
    return side, s_tab


def _decode(o_i1, o_ib2, o_vi, step: int):
    o_v = o_vi[:, 0:32].view(np.float32)
    o_i3b = o_vi[:, 32:64]
    vals = np.zeros((ROWS, VK), np.float32)
    vocab = np.zeros((ROWS, VK), np.int32)
    beams = np.zeros((ROWS, VK), np.int32)
    for t in range(ROWS):
        cand = []  # (val, beam, vocab)
        seen = set()
        vrow = o_v[t]
        exhausted = True  # capture covered everything down to the padding
        for s_ in range(32):
            val = vrow[s_]
            if val < -1e37 or not np.isfinite(val):
                break
            pos_b = int(o_i3b[t, s_])          # in [0, 256)
            j, e = divmod(pos_b, GW)
            pos = int(o_ib2[t, j])             # in [0, 512)
            q, jj = divmod(pos, 32)
            if jj < 16:
                col = int(o_i1[t * 16 + q, jj])            # group in [0, NGA)
            else:
                col = NGA + int(o_i1[t * 16 + q, jj])      # group in [NGA, NG)
            b, h = divmod(q, 2)
            v = h * CH0 + col * GW + e
            if v == 1:
                continue  # pad token pulled in raw by the gather
            key = (b, v)
            if key in seen:
                continue  # h-overlap duplicate
            seen.add(key)
            cand.append((val, b, v))
        else:
            exhausted = False  # all 32 captured slots were live candidates
        assert len(cand) >= VK, f"only {len(cand)} unique candidates for row {t}"
        cand.sort(key=lambda c: (-c[0], c[1] * VOCAB + c[2]))
        # guard: if the 16th value ties with the last captured rank and the
        # capture wasn't exhaustive, a tie cluster might extend past the
        # top-32 window -- refuse rather than be silently wrong
        assert exhausted or cand[VK - 1][0] > vrow[31], (
            f"tie cluster may straddle the top-32 capture for row {t}"
        )
        for k in range(VK):
            vals[t, k] = cand[k][0]
            vocab[t, k] = cand[k][2]
            beams[t, k] = 0 if step == 0 else cand[k][1]
    return vals, vocab, beams


def _run(lprobs: np.ndarray, scores: np.ndarray, step: int, trace: bool = False):
    from concourse.bass_utils import run_bass_kernel_spmd

    nc = _get_nc()
    in_maps = []
    for c in range(NCORES):
        shard = np.ascontiguousarray(lprobs[c * ROWS:(c + 1) * ROWS])
        side, s_tab = _side_inputs(scores[c * ROWS:(c + 1) * ROWS], step)
        in_maps.append({"x": shard, "side": side, "s_tab": s_tab})
    res = run_bass_kernel_spmd(nc, in_maps, core_ids=list(range(NCORES)), trace=trace)
    return res


def kernel(lprobs, scores, step):
    lprobs = np.asarray(lprobs, dtype=np.float32)
    scores = np.asarray(scores, dtype=np.float32)
    step = int(step)

    res = _run(lprobs, scores, step)

    scores_buf = np.zeros((BSZ, VK), np.float32)
    indices_buf = np.zeros((BSZ, VK), np.int32)
    beams_buf = np.zeros((BSZ, VK), np.int32)
    for c in range(NCORES):
        o = res.results[c]
        v, vi, bi = _decode(o["o_i1"], o["o_ib2"], o["o_vi"], step)
        rows = slice(c * ROWS, (c + 1) * ROWS)
        scores_buf[rows] = v
        indices_buf[rows] = vi
        beams_buf[rows] = bi
    return scores_buf, indices_buf, beams_buf
